# revision 17
# baseline (speedup 1.0000x reference)
"""Trainium2 Bass kernel for DequantingLinear (GGML Q8_0 dequant + linear).

Computes out[4096, 12288] = x[4096, 3072] @ dequant(w_q, w_scales).T + bias
where w_q is int32 (int8-valued) with per-32-element-block fp32 scales.

Sharding: tensor-parallel over output features across 8 NeuronCores. Each
core gets the full x and a 1536-row shard of w_q / w_scales / bias,
computes its [4096, 1536] output slice; the host concatenates on axis 1.

Per-core kernel (Tile framework), v5 — zero DMA transposes:
  * DMA xbar transposes were measured to stall the whole DMA complex
    (~5us wall damage per op); all transposes run on the PE instead
    (matmul-with-identity, bf16, batched 4 per PSUM tile) and are
    drained PSUM->SBUF by the otherwise-idle scalar engine (ACT).
  * x path: SWDGE cast-loads (DRAM fp32 -> SBUF bf16, gpsimd ring),
    two 128-token row tiles per op; PE transposes into per-(block,m)
    xt tiles [128, 24, 128].
  * w path: w_q int32 row-chunks via SWDGE, DVE dequant (int32 x
    block-broadcast fp32 scales -> bf16, exact for |q|<=127), PE
    transposes into the resident wt[in-part, k, out] tensor.
  * GEMM: psum[128 tok, 512 out] accumulates 24 bf16 k-tile matmuls,
    k-inner/n-inner so 3 MMs share each stationary x-tile; a
    post-compile pass drops the redundant LDWEIGHTS the legalizer
    emits per-matmul (~78ns each). Bias is added during the
    PSUM->SBUF copy on the vector engine.
  * Phase-1 (n=0 for the first two token blocks) fills the pipeline
    while w chunks 4-11 stream.
  HBM traffic/core: x 50.3 MB + w_q 18.9 MB + out 25.2 MB (+0.6).
"""

import sys

for _p in ("/opt/trn_rl_repo",):
    if _p not in sys.path:
        sys.path.append(_p)

from contextlib import ExitStack

import numpy as np

import concourse.bacc as bacc
import concourse.bass as bass
import concourse.mybir as mybir
from concourse import tile
from concourse.tile_rust import add_dep_helper
from concourse.bass_utils import run_bass_kernel_spmd

FP32 = mybir.dt.float32
BF16 = mybir.dt.bfloat16
INT32 = mybir.dt.int32

N_CORES = 8
TOK, IN, OUT = 4096, 3072, 12288
QK = 32
OUT_SH = OUT // N_CORES
TOK_BLK = 512
NCOL = 512
NB1 = 2


def _build(nc: bass.Bass, repeats: int = 1, serialize: bool = False, mode: str = "full"):
    P = 128
    KT = IN // P
    NBLK = TOK // TOK_BLK
    MT = TOK_BLK // P
    NT = OUT_SH // NCOL
    NB = IN // QK
    OT = OUT_SH // P
    TB = 8  # transposes batched per PSUM tile (bf16 [128,1024] = 1 bank)

    x = nc.dram_tensor("x", [TOK, IN], FP32, kind="ExternalInput")
    w_q = nc.dram_tensor("w_q", [OUT_SH, IN], INT32, kind="ExternalInput")
    w_scales = nc.dram_tensor("w_scales", [OUT_SH, NB], FP32, kind="ExternalInput")
    bias = nc.dram_tensor("bias", [OUT_SH], FP32, kind="ExternalInput")
    ident = nc.dram_tensor("ident", [P, P], BF16, kind="ExternalInput")
    out = nc.dram_tensor("out", [TOK, OUT_SH], FP32, kind="ExternalOutput")
    stub_x = mode in ("gemm", "wonly")
    stub_w = mode in ("gemm", "xonly")
    do_gemm = mode not in ("prep",)
    xsrc = nc.dram_tensor("xsrc", [TOK, IN], BF16) if (stub_x or stub_w) else None

    prev_last = None
    with tile.TileContext(nc) as tc:
      for _rep in range(repeats):
       with ExitStack() as ctx:
        const_pool = ctx.enter_context(tc.tile_pool(name=f"const{_rep}", bufs=1))
        wq_pool = ctx.enter_context(tc.tile_pool(name=f"wq{_rep}", bufs=2))
        wd_pool = ctx.enter_context(tc.tile_pool(name=f"wd{_rep}", bufs=2))
        wt_pool = ctx.enter_context(tc.tile_pool(name=f"wt{_rep}", bufs=1))
        xb_pool = ctx.enter_context(tc.tile_pool(name=f"xb{_rep}", bufs=2))
        xt_pool = ctx.enter_context(tc.tile_pool(name=f"xt{_rep}", bufs=8))
        out_pool = ctx.enter_context(tc.tile_pool(name=f"out{_rep}", bufs=2))
        psum_pool = ctx.enter_context(
            tc.tile_pool(name=f"psum{_rep}", bufs=6, space="PSUM")
        )
        pst_pool = ctx.enter_context(
            tc.tile_pool(name=f"pst{_rep}", bufs=2, space="PSUM")
        )

        entries = []

        idt = const_pool.tile([P, P], BF16, tag="idt")
        entries.append(nc.sync.dma_start(idt[:], ident.ap()[:, :]))

        sc_tiles = []
        if not stub_w:
            for o in range(OT):
                sct = const_pool.tile([P, NB], FP32, tag=f"sc_{o}")
                entries.append(
                    nc.sync.dma_start(sct[:], w_scales.ap()[o * P : (o + 1) * P, :])
                )
                sc_tiles.append(sct)

        bias_rep = const_pool.tile([P, OUT_SH], FP32, tag="bias_rep")
        entries.append(
            nc.sync.dma_start(
                bias_rep[:], bias.ap().unsqueeze(0).to_broadcast([P, OUT_SH])
            )
        )

        wt = wt_pool.tile([P, KT, OUT_SH], BF16, tag="wt")
        last_w_prep = None
        last_x_prep = None

        def pe_transpose(dst3, src2d_slices):
            """Transpose KT [128,128] bf16 slices into dst3 [128, KT, 128],
            batching TB per bf16 PSUM tile with one ACT drain each."""
            res = None
            for k0 in range(0, len(src2d_slices), TB):
                nb2 = min(TB, len(src2d_slices) - k0)
                pst = pst_pool.tile([P, TB * P], BF16, tag="pst")
                for j in range(nb2):
                    nc.tensor.matmul(
                        pst[:, j * P : (j + 1) * P],
                        src2d_slices[k0 + j],
                        idt[:],
                        is_transpose=True,
                        skip_group_check=True,
                    )
                res = nc.scalar.copy(
                    dst3[:, k0 : k0 + nb2, :],
                    pst[:, 0 : nb2 * P].rearrange("p (k q) -> p k q", q=P),
                )
            return res

        def w_chunk(o):
            nonlocal last_w_prep
            rows = slice(o * P, (o + 1) * P)
            if stub_w:
                last_w_prep = nc.sync.dma_start(
                    wt[:, 0, o * P : (o + 1) * P], xsrc.ap()[o * P : (o + 1) * P, 0:P]
                )
                return
            wd = wd_pool.tile([P, IN], BF16, tag="wd")
            wq_i = wq_pool.tile([P, IN], INT32, tag="wq")
            entries.append(nc.gpsimd.dma_start(wq_i[:], w_q.ap()[rows, :]))
            nc.vector.tensor_mul(
                wd[:].rearrange("p (b q) -> p b q", q=QK),
                wq_i[:].rearrange("p (b q) -> p b q", q=QK),
                sc_tiles[o][:].unsqueeze(2).to_broadcast([P, NB, QK]),
            )
            last_w_prep = pe_transpose(
                wt[:, :, o * P : (o + 1) * P],
                [wd[:, k * P : (k + 1) * P] for k in range(KT)],
            )

        xb_tiles = {}

        def load_xpair(b, m0):
            """SWDGE cast-load of two 128-token row tiles in one DMA."""
            tok0 = b * TOK_BLK + m0 * P
            xb = xb_pool.tile([P, 2, IN], BF16, tag="xb")
            entries.append(
                nc.gpsimd.dma_start(
                    xb[:, :, :],
                    x.ap()[tok0 : tok0 + 2 * P, :].rearrange("(u p) i -> p u i", u=2),
                )
            )
            xb_tiles[(b, m0)] = xb

        def load_xt(b, m):
            nonlocal last_x_prep
            tok0 = b * TOK_BLK + m * P
            xt_m = xt_pool.tile([P, KT, P], BF16, tag="xt")
            if stub_x:
                entries.append(
                    nc.sync.dma_start(xt_m[:, 0, :], xsrc.ap()[tok0 : tok0 + P, 0:P])
                )
                return xt_m
            m0 = (m // 2) * 2
            if (b, m0) not in xb_tiles:
                load_xpair(b, m0)
            xb = xb_tiles[(b, m0)]
            u = m - m0
            last_x_prep = pe_transpose(
                xt_m, [xb[:, u, k * P : (k + 1) * P] for k in range(KT)]
            )
            if u == 1:
                xb_tiles.pop((b, m0))
            return xt_m

        # Head: w chunks interleaved with phase-1 x tiles on the SWDGE ring.
        xt_tiles = {}
        for o in range(4):
            w_chunk(o)
        for m in range(MT):
            xt_tiles[(0, m)] = load_xt(0, m)
        for o in range(4, 8):
            w_chunk(o)
        for m in range(MT):
            xt_tiles[(1, m)] = load_xt(1, m)
        for o in range(8, OT):
            w_chunk(o)

        def gemm_mgroup(xt_m, b, m, ns):
            if not do_gemm:
                return None
            tok0 = b * TOK_BLK + m * P
            pss = []
            for n in ns:
                ps_n = psum_pool.tile([P, NCOL], FP32, tag="ps")
                pss.append(ps_n)
            for k in range(KT):
                for i, n in enumerate(ns):
                    nc.tensor.matmul(
                        pss[i][:],
                        xt_m[:, k, :],
                        wt[:, k, n * NCOL : (n + 1) * NCOL],
                        start=(k == 0),
                        stop=(k == KT - 1),
                    )
            ob = out_pool.tile([P, NT * NCOL], FP32, tag="ob")
            for i, n in enumerate(ns):
                nc.vector.tensor_add(
                    ob[:, i * NCOL : (i + 1) * NCOL],
                    pss[i][:],
                    bias_rep[:, n * NCOL : (n + 1) * NCOL],
                )
            n0 = ns[0]
            return nc.sync.dma_start(
                out.ap()[tok0 : tok0 + P, n0 * NCOL : (n0 + len(ns)) * NCOL],
                ob[:, 0 : len(ns) * NCOL],
            )

        # Phase-1 GEMM: n=0 for the first NB1 blocks.
        for b in range(NB1):
            for m in range(MT):
                gemm_mgroup(xt_tiles[(b, m)], b, m, [0])

        # Main loop with one-block x prefetch.
        last_store = None
        for b in range(NBLK):
            nb_ = b + 1
            if NB1 <= nb_ < NBLK:
                for m in range(MT):
                    xt_tiles[(nb_, m)] = load_xt(nb_, m)
            for m in range(MT):
                xt_m = xt_tiles.pop((b, m))
                ns = [n for n in range(NT) if not (b < NB1 and n == 0)]
                last_store = gemm_mgroup(xt_m, b, m, ns)

        if not do_gemm:
            last_store = last_x_prep
        if serialize and prev_last is not None:
            for e in entries:
                add_dep_helper(e.ins, prev_last.ins, reason="serialize reps")
        prev_last = last_store
    return nc


def _dedup_ldweights(nc):
    """Drop InstLdweights that reload the exact weights already resident in
    the PE array (sync-free ones only). The legalizer emits one LDW per
    matmul with no dedup; k-inner/n-inner ordering makes 2/3 redundant."""
    removed = 0
    for fn in nc.m.functions:
        for bb in fn.blocks:
            insts = list(bb.instructions)
            new, last_fp = [], None
            for i in insts:
                if type(i).__name__ == "InstLdweights":
                    si = i.sync_info
                    clean = si is None or (
                        len(si.on_wait) == 0 and len(si.on_update) == 0
                    )
                    fp = str(i.ins[-1])
                    if clean and fp == last_fp:
                        removed += 1
                        continue
                    last_fp = fp
                new.append(i)
            if len(new) != len(insts):
                bb.instructions = new
    return removed


def _compile(nc):
    nc.compile()
    _dedup_ldweights(nc)
    return nc


_COMPILED_NC = None


def _get_nc():
    global _COMPILED_NC
    if _COMPILED_NC is None:
        nc = bacc.Bacc("TRN2", target_bir_lowering=False, debug=False)
        _build(nc)
        _compile(nc)
        _COMPILED_NC = nc
    return _COMPILED_NC


def kernel(x, w_q, w_scales, bias):
    assert x.shape == (TOK, IN) and w_q.shape == (OUT, IN)
    import ml_dtypes

    nc = _get_nc()
    x = np.ascontiguousarray(np.asarray(x, dtype=np.float32))
    w_q = np.asarray(w_q, dtype=np.int32)
    w_scales = np.asarray(w_scales, dtype=np.float32)
    bias = np.asarray(bias, dtype=np.float32)
    ident = np.eye(128, dtype=ml_dtypes.bfloat16)
    in_maps = []
    for c in range(N_CORES):
        r = slice(c * OUT_SH, (c + 1) * OUT_SH)
        in_maps.append(
            {
                "x": x,
                "w_q": np.ascontiguousarray(w_q[r]),
                "w_scales": np.ascontiguousarray(w_scales[r]),
                "bias": np.ascontiguousarray(bias[r]),
                "ident": ident,
            }
        )
    res = run_bass_kernel_spmd(nc, in_maps, list(range(N_CORES)))
    return np.concatenate([res.results[c]["out"] for c in range(N_CORES)], axis=1)
